# revision 16
# baseline (speedup 1.0000x reference)
"""GQA attention block on 8 trn2 NeuronCores.

Sharding: core c = (batch b=c//4, kv-head-pair g=c%4). Each core owns kv heads
{2g, 2g+1} and their 8 query heads (GQA tile mapping: q-head i -> kv-head i%8),
with Wq/Wk/Wv column-sharded and Wo row-sharded; host sums the 4 partial
outputs per batch and adds bo.

Device strategy (per core):
  - scores: the two heads of a pair run CONCURRENTLY as 64x128 PE row tiles
    (T0 reads SBUF partitions 0-63 = even head, T8 reads 64-127 = odd head),
    each writing its own PSUM bank of a shared [128,1024] f32 quad (a PSUM
    bank must never be written by two row tiles concurrently).
  - exp on ACT in 1024-wide chunks (amortizes the ~352-cycle ACTIVATE
    overhead), scale=1/8 folded in, bf16 out.
  - AV and all projections run as plain full-array 128x128 matmuls (single
    writer per PSUM bank). AV: lhsT = Vp chunk [128 kpos, 65] with a ones
    column giving the softmax denominator in psum row 64.
  - Q/O projection work is emitted as fine-grained "filler" units between
    attention steps so the PE stays busy while ACT chews exp.
  - RoPE: PSUM evacuated to bf16 SBUF, rotate_half via partition-shifted DVE
    copies, cos/sin combine in bf16 (fast DVE modes).
"""

import os
from contextlib import ExitStack

import numpy as np
import ml_dtypes

D = 2048
QH = 32
KVH = 8
HD = 64
B = 2
S = 2048
THETA = 1000000.0
P = 128
NCORES = 8

BF16 = ml_dtypes.bfloat16

_CACHE = {}


def _build_program():
    import concourse.bass as bass
    import concourse.tile as tile
    from concourse import bacc, mybir

    nc = bacc.Bacc(
        "TRN2",
        target_bir_lowering=False,
        debug=False,
        enable_asserts=False,
        num_devices=NCORES,
    )
    bf = mybir.dt.bfloat16
    f32 = mybir.dt.float32

    qT = nc.dram_tensor("qT", [D, S], bf, kind="ExternalInput").ap()
    kT = nc.dram_tensor("kT", [D, S], bf, kind="ExternalInput").ap()
    vT = nc.dram_tensor("vT", [D, S], bf, kind="ExternalInput").ap()
    wqt = nc.dram_tensor("wqt", [D, 512], bf, kind="ExternalInput").ap()
    wkt = nc.dram_tensor("wkt", [D, 128], bf, kind="ExternalInput").ap()
    wv = nc.dram_tensor("wv", [D, 128], bf, kind="ExternalInput").ap()
    wo = nc.dram_tensor("wo", [512, D], bf, kind="ExternalInput").ap()
    cosr = nc.dram_tensor("cosr", [P, S], bf, kind="ExternalInput").ap()
    sinr = nc.dram_tensor("sinr", [P, S], bf, kind="ExternalInput").ap()
    out = nc.dram_tensor("out", [S, D], f32, kind="ExternalOutput").ap()

    # partitioned DRAM views
    qT3 = qT.rearrange("(o p) s -> p o s", p=P)    # [128, 16, 2048]
    kT3 = kT.rearrange("(o p) s -> p o s", p=P)
    vT3 = vT.rearrange("(o p) s -> p o s", p=P)
    wqt3 = wqt.rearrange("(o p) m -> p o m", p=P)  # [128, 16, 512]
    wkt3 = wkt.rearrange("(o p) m -> p o m", p=P)  # [128, 16, 128]
    wv3 = wv.rearrange("(o p) m -> p o m", p=P)    # [128, 16, 128]
    wo3 = wo.rearrange("(o p) d -> p o d", p=P)    # [128, 4, 2048]
    out3 = out.rearrange("(t p) d -> p t d", p=P)  # [128, 16, 2048]

    scale = 1.0 / float(np.sqrt(HD))
    LO = slice(0, 64)
    HI = slice(64, 128)

    with tile.TileContext(nc) as tc, ExitStack() as ctx:
        Exp = mybir.ActivationFunctionType.Exp
        const = ctx.enter_context(tc.tile_pool(name="const", bufs=1))
        persist = ctx.enter_context(tc.tile_pool(name="persist", bufs=1))
        qpt_pool = ctx.enter_context(tc.tile_pool(name="qptp", bufs=2))
        outT_pool = ctx.enter_context(tc.tile_pool(name="outTp", bufs=2))
        vkin = ctx.enter_context(tc.tile_pool(name="vkin", bufs=2))
        qin = ctx.enter_context(tc.tile_pool(name="qin", bufs=2))
        rtmp = ctx.enter_context(tc.tile_pool(name="rtmp", bufs=2))
        fout = ctx.enter_context(tc.tile_pool(name="fout", bufs=5))
        ntmp = ctx.enter_context(tc.tile_pool(name="ntmp", bufs=2))
        etp = ctx.enter_context(tc.tile_pool(name="etp", bufs=3))
        qpsum = ctx.enter_context(tc.tile_pool(name="qpsum", bufs=2, space="PSUM"))
        apsum = ctx.enter_context(tc.tile_pool(name="apsum", bufs=2, space="PSUM"))
        ppsum = ctx.enter_context(tc.tile_pool(name="ppsum", bufs=1, space="PSUM"))

        # ---- resident weights / tables (small V/K weights first so the
        # V projection can start while the big tables stream in) ----
        wv_sb = const.tile([P, 16, 128], bf, tag="wv")
        nc.sync.dma_start(wv_sb[:], wv3[:])
        wkt_sb = const.tile([P, 16, 128], bf, tag="wkt")
        nc.sync.dma_start(wkt_sb[:], wkt3[:])
        cos_sb = const.tile([P, S], bf, tag="cos")
        nc.sync.dma_start(cos_sb[:], cosr[:])
        sin_sb = const.tile([P, S], bf, tag="sin")
        nc.sync.dma_start(sin_sb[:], sinr[:])
        wqt_sb = const.tile([P, 16, 512], bf, tag="wqt")
        nc.sync.dma_start(wqt_sb[:], wqt3[:])
        wo_sb = const.tile([P, 4, 2048], bf, tag="wo")

        # ---- persistent intermediates ----
        kpt_b = persist.tile([P, S], bf, tag="kpt")       # rotated K^T pair-stacked
        vp_sb = persist.tile([P, 16, 130], bf, tag="vp")  # Vp + ones cols
        nc.vector.memset(vp_sb[:, :, 64:65], 1.0)
        nc.vector.memset(vp_sb[:, :, 129:130], 1.0)

        def rope(ps, gs, dst):
            """RoPE: ps [128,512] f32 psum (pair-stacked head dims) ->
            dst bf16 [128,512]. Evacuate early to free the bank, then bf16
            DVE ops (rotate_half = partition-shifted copies)."""
            ev0 = rtmp.tile([P, 512], bf, tag="ev0", name="ev0")
            nc.vector.tensor_copy(out=ev0[:], in_=ps[:, 0:512])
            ev = rtmp.tile([P, 512], bf, tag="ev", name="ev")
            nc.vector.tensor_add(out=ev[:], in0=ev0[:], in1=ps[:, 512:1024])
            rot = rtmp.tile([P, 512], bf, tag="rot", name="rot")
            for b0 in (0, 64):
                nc.vector.tensor_scalar_mul(
                    rot[b0 : b0 + 32, :], ev[b0 + 32 : b0 + 64, :], -1.0
                )
                nc.vector.tensor_copy(
                    out=rot[b0 + 32 : b0 + 64, :], in_=ev[b0 : b0 + 32, :]
                )
            t1 = rtmp.tile([P, 512], bf, tag="t1", name="t1")
            t2 = rtmp.tile([P, 512], bf, tag="t2", name="t2")
            nc.vector.tensor_mul(out=t1[:], in0=ev[:], in1=cos_sb[:, gs])
            nc.vector.tensor_mul(out=t2[:], in0=rot[:], in1=sin_sb[:, gs])
            nc.vector.tensor_add(out=dst, in0=t1[:], in1=t2[:])

        def accum(chunks, nacc, get_lhsT, get_rhs, alloc, consume,
                  prep=None, unit=4, width=None, paired=True):
            """Generator emitting row-tile-paired PSUM accumulation chains.

            For each chunk, tile T0 (SBUF partitions 0-63) accumulates the
            low contraction halves into psum columns [0:N] (bank group A)
            while T8 (partitions 64-127) concurrently accumulates the high
            halves into columns [N:2N] (bank group B). Each bank has a
            single writer; the consumer merges A+B. Yields every `unit`
            matmuls."""
            cnt = 0
            for desc in list(chunks):
                if prep is not None:
                    prep(desc)
                ps = alloc(desc)
                w = width if width is not None else 512
                for o in range(nacc):
                    lhsT = get_lhsT(desc, o)
                    rhs = get_rhs(desc, o)
                    if paired:
                        nc.tensor.matmul(
                            ps[:, 0:w], lhsT=lhsT[LO], rhs=rhs[LO],
                            start=(o == 0), stop=(o == nacc - 1),
                        )
                        nc.tensor.matmul(
                            ps[:, 512 : 512 + w], lhsT=lhsT[HI], rhs=rhs[HI],
                            start=(o == 0), stop=(o == nacc - 1),
                        )
                        cnt += 2
                    else:
                        nc.tensor.matmul(
                            ps[:, 0:w], lhsT=lhsT, rhs=rhs,
                            start=(o == 0), stop=(o == nacc - 1),
                        )
                        cnt += 1
                    if cnt >= unit:
                        cnt = 0
                        yield
                consume(ps, desc)

        def run_all(gen):
            for _ in gen:
                pass

        # ================= prologue: V projection =================
        # vp[s,128] per s-tile via stationary vT s-tiles, moving wv.
        def v_phase():

            def prep(st):
                if st % 4 != 0:
                    return
                sc = st // 4
                for c in (sc, sc + 1):
                    if c < 4 and c not in vh_prefetch:
                        load_vh(c)

            def alloc(st):
                return qpsum.tile([P, 1024], f32, tag="qp", name="psv")

            def get_lhsT(st, o):
                return vh_prefetch[st // 4][
                    :, o, (st % 4) * 128 : (st % 4 + 1) * 128
                ]

            def get_rhs(st, o):
                return wv_sb[:, o, :]

            def consume(ps, st):
                nc.vector.tensor_copy(out=vp_sb[:, st, 0:64], in_=ps[:, 0:64])
                nc.vector.tensor_add(
                    out=vp_sb[:, st, 0:64], in0=vp_sb[:, st, 0:64],
                    in1=ps[:, 512:576],
                )
                nc.vector.tensor_copy(out=vp_sb[:, st, 65:129], in_=ps[:, 64:128])
                nc.vector.tensor_add(
                    out=vp_sb[:, st, 65:129], in0=vp_sb[:, st, 65:129],
                    in1=ps[:, 576:640],
                )

            return accum(range(16), 16, get_lhsT, get_rhs, alloc, consume,
                         prep=prep, width=128)

        # prefetch the first K and V chunks (their projections and the rest
        # of the V/K work run as filler inside the attention stream)
        kh_prefetch = {}
        vh_prefetch = {}

        def load_kh(ns):
            kh = vkin.tile([P, 16, 512], bf, tag="kh", name="kh")
            for o4 in range(0, 16, 4):
                nc.sync.dma_start(
                    kh[:, o4 : o4 + 4, :],
                    kT3[:, o4 : o4 + 4, ns * 512 : (ns + 1) * 512],
                )
            kh_prefetch[ns] = kh

        def load_vh(sc):
            vh = vkin.tile([P, 16, 512], bf, tag="vh", name="vh")
            for o4 in range(0, 16, 4):
                nc.sync.dma_start(
                    vh[:, o4 : o4 + 4, :],
                    vT3[:, o4 : o4 + 4, sc * 512 : (sc + 1) * 512],
                )
            vh_prefetch[sc] = vh

        load_kh(0)

        # ================= prologue: K projection + RoPE =================
        def k_phase(chunks):
            def prep(ns):
                for c in (ns, ns + 1):
                    if c < 4 and c not in kh_prefetch:
                        load_kh(c)

            def alloc(ns):
                return ppsum.tile([P, 1024], f32, tag="pp", name="psk")

            def get_lhsT(ns, o):
                return wkt_sb[:, o, :]

            def get_rhs(ns, o):
                return kh_prefetch[ns][:, o, :]

            def consume(ps, ns):
                gs = slice(ns * 512, (ns + 1) * 512)
                rope(ps, gs, kpt_b[:, gs])

            return accum(chunks, 16, get_lhsT, get_rhs, alloc, consume,
                         prep=prep)

        # ================= Q projection (one quarter) =================
        qpt_tiles = {}

        def load_qh(quarter):
            qh_sb = qin.tile([P, 16, 512], bf, tag="qin", name="qh")
            for o4 in range(0, 16, 4):
                nc.sync.dma_start(
                    qh_sb[:, o4 : o4 + 4, :],
                    qT3[:, o4 : o4 + 4, quarter * 512 : (quarter + 1) * 512],
                )
            return qh_sb

        def qproj_gen(quarter, qh_sb):
            gs = slice(quarter * 512, (quarter + 1) * 512)
            qpt_tiles[quarter] = qpt_pool.tile([P, 4, 512], bf, tag="qpt", name="qpt_q")

            def alloc(m):
                return ppsum.tile([P, 1024], f32, tag="pp", name="psq")

            def get_lhsT(m, o):
                return wqt_sb[:, o, m * 128 : (m + 1) * 128]

            def get_rhs(m, o):
                return qh_sb[:, o, :]

            def consume(ps, m):
                rope(ps, gs, qpt_tiles[quarter][:, m, :])

            return accum(range(4), 16, get_lhsT, get_rhs, alloc, consume)

        # ================= output projection (one quarter) =================
        outT_tiles = {}

        def outproj_gen(quarter):
            combos = [(qi, dn) for qi in range(4) for dn in range(4)]
            outT_q = outT_tiles[quarter]

            def alloc(c):
                return ppsum.tile([P, 1024], f32, tag="pp", name="psf")

            def get_lhsT(c, o):
                qi, dn = c
                return outT_q[:, o, qi * 128 : (qi + 1) * 128]

            def get_rhs(c, o):
                qi, dn = c
                return wo_sb[:, o, dn * 512 : (dn + 1) * 512]

            def consume(ps, c):
                qi, dn = c
                of = fout.tile([P, 512], f32, tag="of", name="of")
                nc.vector.tensor_copy(out=of[:], in_=ps[:, 0:512])
                nc.sync.dma_start(
                    out3[:, quarter * 4 + qi, dn * 512 : (dn + 1) * 512], of[:]
                )

            return accum(combos, 4, get_lhsT, get_rhs, alloc, consume,
                         paired=False, unit=2)

        # ---- prologue: only K chunk 0 and quarter-0 Q projection run up
        # front (minimum needed for the first scores); V and the remaining K
        # chunks stream in as filler during early attention ----
        qh0 = load_qh(0)
        load_vh(0)
        load_kh(1)
        load_vh(1)
        nc.sync.dma_start(wo_sb[:], wo3[:])
        pro = [k_phase([0, 1, 2, 3]), v_phase(), qproj_gen(0, qh0)]
        weights = [1, 2, 1]
        while pro:
            for g, w in zip(list(pro), list(weights)):
                try:
                    for _ in range(w):
                        next(g)
                except StopIteration:
                    i = pro.index(g)
                    pro.pop(i)
                    weights.pop(i)

        # ================= main loop: flattened attention pipeline =========
        from collections import deque

        fillerq = deque()

        def pop_filler():
            while fillerq:
                try:
                    next(fillerq[0])
                    return
                except StopIteration:
                    fillerq.popleft()

        av_tiles = {}

        def av_step(q, pr, pet, pkt, last):
            first = pkt == 0
            if first:
                av0 = apsum.tile([65, 512], f32, tag="av", name="av0")
                av1 = apsum.tile([65, 512], f32, tag="av", name="av1")
                av_tiles[(q, pr)] = (av0, av1)
            av0, av1 = av_tiles[(q, pr)]
            nc.tensor.matmul(
                av0, lhsT=vp_sb[:, pkt, 0:65], rhs=pet[:, 0:512],
                start=first, stop=last,
            )
            nc.tensor.matmul(
                av1, lhsT=vp_sb[:, pkt, 65:130], rhs=pet[:, 512:1024],
                start=first, stop=last,
            )
            if last:
                outT_q = outT_tiles[q]
                avcs = []
                for av in (av0, av1):
                    avc = ntmp.tile([65, 512], f32, tag="avc", name="avc")
                    nc.vector.tensor_copy(out=avc[:], in_=av[:])
                    avcs.append(avc)
                for e, avc in enumerate(avcs):
                    recip = ntmp.tile([1, 512], f32, tag="recip", name="recip")
                    nc.vector.reciprocal(recip[:], avc[64:65, :])
                    bc = ntmp.tile([64, 512], f32, tag="bc", name="bc")
                    nc.gpsimd.partition_broadcast(bc[:], recip[:])
                    hp = slice(e * 64, e * 64 + 64)
                    nc.vector.tensor_mul(
                        out=outT_q[hp, pr, :], in0=avcs[e][0:64, :], in1=bc[:]
                    )

        steps = [(q, pr, kt) for q in range(4) for pr in range(4)
                 for kt in range(16)]
        prev = None
        for q, pr, kt in steps:
            if pr == 0 and kt == 0:
                outT_q = outT_pool.tile([P, 4, 512], bf, tag="outT",
                                        name="outT_q")
                outT_tiles[q] = outT_q
                if q < 3:
                    qh_next = load_qh(q + 1)
                    fillerq.append(qproj_gen(q + 1, qh_next))
            if pr == 1 and kt == 0 and q > 0:
                # deferred so outT(q-1)'s last norm (emitted during pair 0's
                # first steps) exists before any outproj unit references it
                fillerq.append(outproj_gen(q - 1))

            qpt_q = qpt_tiles[q]
            quad = qpsum.tile([P, 1024], f32, tag="qp", name="quad")
            ksl = slice(kt * 128, (kt + 1) * 128)
            nc.tensor.matmul(
                quad[:, 0:512], lhsT=kpt_b[LO, ksl],
                rhs=qpt_q[LO, pr, :], start=True, stop=True,
            )
            nc.tensor.matmul(
                quad[:, 512:1024], lhsT=kpt_b[HI, ksl],
                rhs=qpt_q[HI, pr, :], start=True, stop=True,
            )
            et = etp.tile([P, 1024], bf, tag="et", name="et")
            nc.scalar.activation(
                out=et[:], in_=quad[:], func=Exp, scale=scale
            )
            # pops come BEFORE the AV step: the V/K filler units produce the
            # vp/kpt tiles that AV and scores consume, and a filler matmul
            # queued behind a stalled AV would deadlock the PE FIFO
            pop_filler()
            if prev is not None:
                pq, ppr, pet, pkt = prev
                av_step(pq, ppr, pet, pkt, last=(pkt == 15))
            prev = (q, pr, et, kt)
        # drain the last AV step and remaining filler
        pq, ppr, pet, pkt = prev
        av_step(pq, ppr, pet, pkt, last=True)
        while fillerq:
            try:
                next(fillerq[0])
            except StopIteration:
                fillerq.popleft()

        # epilogue: out projection of last quarter
        run_all(outproj_gen(3))

    nc.finalize()
    return nc


def _host_inputs(q, k, v, Wq, Wk, Wv, Wo):
    """Build the 8 per-core input dicts."""
    inv_freq = 1.0 / (THETA ** (np.arange(0, HD, 2, dtype=np.float32) / HD))
    t = np.arange(S, dtype=np.float32)
    freqs = np.einsum("i,j->ij", t, inv_freq)
    emb = np.concatenate([freqs, freqs], axis=-1)  # [S, 64]
    cosT = np.ascontiguousarray(np.cos(emb).T, dtype=np.float32)  # [64, S]
    sinT = np.ascontiguousarray(np.sin(emb).T, dtype=np.float32)
    cos_rep = np.concatenate([cosT, cosT], axis=0).astype(BF16)  # [128, S]
    sin_rep = np.concatenate([sinT, sinT], axis=0).astype(BF16)

    qT = [np.ascontiguousarray(q[b].T).astype(BF16) for b in range(B)]
    kTt = [np.ascontiguousarray(k[b].T).astype(BF16) for b in range(B)]
    vTt = [np.ascontiguousarray(v[b].T).astype(BF16) for b in range(B)]

    in_maps = []
    for c in range(NCORES):
        b, g = divmod(c, 4)
        # pair-interleaved: chunk i of 128 cols = (kv0 q-head i, kv1 q-head i)
        qheads = [2 * g, 2 * g + 1, 2 * g + 8, 2 * g + 9,
                  2 * g + 16, 2 * g + 17, 2 * g + 24, 2 * g + 25]
        qcols = np.concatenate([np.arange(h * HD, (h + 1) * HD) for h in qheads])
        kvcols = np.arange(2 * g * HD, (2 * g + 2) * HD)

        wqt_np = np.ascontiguousarray(Wq[:, qcols]).astype(BF16)
        wkt_np = np.ascontiguousarray(Wk[:, kvcols]).astype(BF16)
        wv_np = np.ascontiguousarray(Wv[:, kvcols]).astype(BF16)
        wo_np = np.ascontiguousarray(Wo[qcols, :]).astype(BF16)

        in_maps.append({
            "qT": qT[b], "kT": kTt[b], "vT": vTt[b],
            "wqt": wqt_np, "wkt": wkt_np, "wv": wv_np, "wo": wo_np,
            "cosr": cos_rep, "sinr": sin_rep,
        })
    return in_maps


def kernel(q, k, v, attn_mask, Wq, Wk, Wv, Wo, bo):
    from concourse.bass_utils import run_bass_kernel_spmd

    q = np.asarray(q, dtype=np.float32)
    k = np.asarray(k, dtype=np.float32)
    v = np.asarray(v, dtype=np.float32)
    Wq = np.asarray(Wq, dtype=np.float32)
    Wk = np.asarray(Wk, dtype=np.float32)
    Wv = np.asarray(Wv, dtype=np.float32)
    Wo = np.asarray(Wo, dtype=np.float32)
    bo = np.asarray(bo, dtype=np.float32)

    if "nc" not in _CACHE:
        _CACHE["nc"] = _build_program()
    nc = _CACHE["nc"]

    in_maps = _host_inputs(q, k, v, Wq, Wk, Wv, Wo)
    trace = bool(int(os.environ.get("KERNEL_TRACE", "0")))
    res = run_bass_kernel_spmd(nc, in_maps, core_ids=list(range(NCORES)),
                               trace=trace)
    _CACHE["last_result"] = res

    out = np.zeros((B, S, D), dtype=np.float32)
    for c in range(NCORES):
        b = c // 4
        out[b] += np.asarray(res.results[c]["out"], dtype=np.float32)
    out += bo[None, None, :]
    return out


# revision 17
# speedup vs baseline: 1.0038x; 1.0038x over previous
"""GQA attention block on 8 trn2 NeuronCores.

Sharding: core c = (batch b=c//4, kv-head-pair g=c%4). Each core owns kv heads
{2g, 2g+1} and their 8 query heads (GQA tile mapping: q-head i -> kv-head i%8),
with Wq/Wk/Wv column-sharded and Wo row-sharded; host sums the 4 partial
outputs per batch and adds bo.

Device strategy (per core):
  - scores: the two heads of a pair run CONCURRENTLY as 64x128 PE row tiles
    (T0 reads SBUF partitions 0-63 = even head, T8 reads 64-127 = odd head),
    each writing its own PSUM bank of a shared [128,1024] f32 quad (a PSUM
    bank must never be written by two row tiles concurrently).
  - exp on ACT in 1024-wide chunks (amortizes the ~352-cycle ACTIVATE
    overhead), scale=1/8 folded in, bf16 out.
  - AV and all projections run as plain full-array 128x128 matmuls (single
    writer per PSUM bank). AV: lhsT = Vp chunk [128 kpos, 65] with a ones
    column giving the softmax denominator in psum row 64.
  - Q/O projection work is emitted as fine-grained "filler" units between
    attention steps so the PE stays busy while ACT chews exp.
  - RoPE: PSUM evacuated to bf16 SBUF, rotate_half via partition-shifted DVE
    copies, cos/sin combine in bf16 (fast DVE modes).
"""

import os
from contextlib import ExitStack

import numpy as np
import ml_dtypes

D = 2048
QH = 32
KVH = 8
HD = 64
B = 2
S = 2048
THETA = 1000000.0
P = 128
NCORES = 8

BF16 = ml_dtypes.bfloat16

_CACHE = {}


def _build_program():
    import concourse.bass as bass
    import concourse.tile as tile
    from concourse import bacc, mybir

    nc = bacc.Bacc(
        "TRN2",
        target_bir_lowering=False,
        debug=False,
        enable_asserts=False,
        num_devices=NCORES,
    )
    bf = mybir.dt.bfloat16
    f32 = mybir.dt.float32

    qT = nc.dram_tensor("qT", [D, S], bf, kind="ExternalInput").ap()
    kT = nc.dram_tensor("kT", [D, S], bf, kind="ExternalInput").ap()
    vT = nc.dram_tensor("vT", [D, S], bf, kind="ExternalInput").ap()
    wqt = nc.dram_tensor("wqt", [D, 512], bf, kind="ExternalInput").ap()
    wkt = nc.dram_tensor("wkt", [D, 128], bf, kind="ExternalInput").ap()
    wv = nc.dram_tensor("wv", [D, 128], bf, kind="ExternalInput").ap()
    wo = nc.dram_tensor("wo", [512, D], bf, kind="ExternalInput").ap()
    cosr = nc.dram_tensor("cosr", [P, S], bf, kind="ExternalInput").ap()
    sinr = nc.dram_tensor("sinr", [P, S], bf, kind="ExternalInput").ap()
    out = nc.dram_tensor("out", [S, D], f32, kind="ExternalOutput").ap()

    # partitioned DRAM views
    qT3 = qT.rearrange("(o p) s -> p o s", p=P)    # [128, 16, 2048]
    kT3 = kT.rearrange("(o p) s -> p o s", p=P)
    vT3 = vT.rearrange("(o p) s -> p o s", p=P)
    wqt3 = wqt.rearrange("(o p) m -> p o m", p=P)  # [128, 16, 512]
    wkt3 = wkt.rearrange("(o p) m -> p o m", p=P)  # [128, 16, 128]
    wv3 = wv.rearrange("(o p) m -> p o m", p=P)    # [128, 16, 128]
    wo3 = wo.rearrange("(o p) d -> p o d", p=P)    # [128, 4, 2048]
    out3 = out.rearrange("(t p) d -> p t d", p=P)  # [128, 16, 2048]

    scale = 1.0 / float(np.sqrt(HD))
    LO = slice(0, 64)
    HI = slice(64, 128)

    with tile.TileContext(nc) as tc, ExitStack() as ctx:
        Exp = mybir.ActivationFunctionType.Exp
        const = ctx.enter_context(tc.tile_pool(name="const", bufs=1))
        persist = ctx.enter_context(tc.tile_pool(name="persist", bufs=1))
        qpt_pool = ctx.enter_context(tc.tile_pool(name="qptp", bufs=2))
        outT_pool = ctx.enter_context(tc.tile_pool(name="outTp", bufs=2))
        vkin = ctx.enter_context(tc.tile_pool(name="vkin", bufs=2))
        qin = ctx.enter_context(tc.tile_pool(name="qin", bufs=2))
        rtmp = ctx.enter_context(tc.tile_pool(name="rtmp", bufs=2))
        fout = ctx.enter_context(tc.tile_pool(name="fout", bufs=5))
        ntmp = ctx.enter_context(tc.tile_pool(name="ntmp", bufs=2))
        etp = ctx.enter_context(tc.tile_pool(name="etp", bufs=3))
        qpsum = ctx.enter_context(tc.tile_pool(name="qpsum", bufs=2, space="PSUM"))
        apsum = ctx.enter_context(tc.tile_pool(name="apsum", bufs=2, space="PSUM"))
        ppsum = ctx.enter_context(tc.tile_pool(name="ppsum", bufs=1, space="PSUM"))

        # ---- resident weights / tables (small V/K weights first so the
        # V projection can start while the big tables stream in) ----
        wv_sb = const.tile([P, 16, 128], bf, tag="wv")
        nc.sync.dma_start(wv_sb[:], wv3[:])
        wkt_sb = const.tile([P, 16, 128], bf, tag="wkt")
        nc.sync.dma_start(wkt_sb[:], wkt3[:])
        cos_sb = const.tile([P, S], bf, tag="cos")
        nc.sync.dma_start(cos_sb[:], cosr[:])
        sin_sb = const.tile([P, S], bf, tag="sin")
        nc.sync.dma_start(sin_sb[:], sinr[:])
        wqt_sb = const.tile([P, 16, 512], bf, tag="wqt")
        nc.sync.dma_start(wqt_sb[:], wqt3[:])
        wo_sb = const.tile([P, 4, 2048], bf, tag="wo")

        # ---- persistent intermediates ----
        kpt_b = persist.tile([P, S], bf, tag="kpt")       # rotated K^T pair-stacked
        vp_sb = persist.tile([P, 16, 130], bf, tag="vp")  # Vp + ones cols
        nc.vector.memset(vp_sb[:, :, 64:65], 1.0)
        nc.vector.memset(vp_sb[:, :, 129:130], 1.0)

        def rope(ps, gs, dst):
            """RoPE: ps [128,512] f32 psum (pair-stacked head dims) ->
            dst bf16 [128,512]. Evacuate early to free the bank, then bf16
            DVE ops (rotate_half = partition-shifted copies)."""
            ev = rtmp.tile([P, 512], bf, tag="ev", name="ev")
            nc.vector.tensor_copy(out=ev[:], in_=ps[:, 0:512])
            rot = rtmp.tile([P, 512], bf, tag="rot", name="rot")
            for b0 in (0, 64):
                nc.vector.tensor_scalar_mul(
                    rot[b0 : b0 + 32, :], ev[b0 + 32 : b0 + 64, :], -1.0
                )
                nc.vector.tensor_copy(
                    out=rot[b0 + 32 : b0 + 64, :], in_=ev[b0 : b0 + 32, :]
                )
            t1 = rtmp.tile([P, 512], bf, tag="t1", name="t1")
            t2 = rtmp.tile([P, 512], bf, tag="t2", name="t2")
            nc.vector.tensor_mul(out=t1[:], in0=ev[:], in1=cos_sb[:, gs])
            nc.vector.tensor_mul(out=t2[:], in0=rot[:], in1=sin_sb[:, gs])
            nc.vector.tensor_add(out=dst, in0=t1[:], in1=t2[:])

        def accum(chunks, nacc, get_lhsT, get_rhs, alloc, consume,
                  prep=None, unit=4, width=None, paired=True):
            """Generator emitting row-tile-paired PSUM accumulation chains.

            For each chunk, tile T0 (SBUF partitions 0-63) accumulates the
            low contraction halves into psum columns [0:N] (bank group A)
            while T8 (partitions 64-127) concurrently accumulates the high
            halves into columns [N:2N] (bank group B). Each bank has a
            single writer; the consumer merges A+B. Yields every `unit`
            matmuls."""
            cnt = 0
            for desc in list(chunks):
                if prep is not None:
                    prep(desc)
                ps = alloc(desc)
                w = width if width is not None else 512
                for o in range(nacc):
                    lhsT = get_lhsT(desc, o)
                    rhs = get_rhs(desc, o)
                    if paired:
                        nc.tensor.matmul(
                            ps[:, 0:w], lhsT=lhsT[LO], rhs=rhs[LO],
                            start=(o == 0), stop=(o == nacc - 1),
                        )
                        nc.tensor.matmul(
                            ps[:, 512 : 512 + w], lhsT=lhsT[HI], rhs=rhs[HI],
                            start=(o == 0), stop=(o == nacc - 1),
                        )
                        cnt += 2
                    else:
                        nc.tensor.matmul(
                            ps[:, 0:w], lhsT=lhsT, rhs=rhs,
                            start=(o == 0), stop=(o == nacc - 1),
                        )
                        cnt += 1
                    if cnt >= unit:
                        cnt = 0
                        yield
                consume(ps, desc)

        def run_all(gen):
            for _ in gen:
                pass

        # ================= prologue: V projection =================
        # vp[s,128] per s-tile via stationary vT s-tiles, moving wv.
        def v_phase():

            def prep(st):
                if st % 4 != 0:
                    return
                sc = st // 4
                for c in (sc, sc + 1):
                    if c < 4 and c not in vh_prefetch:
                        load_vh(c)

            def alloc(st):
                return qpsum.tile([P, 1024], f32, tag="qp", name="psv")

            def get_lhsT(st, o):
                return vh_prefetch[st // 4][
                    :, o, (st % 4) * 128 : (st % 4 + 1) * 128
                ]

            def get_rhs(st, o):
                return wv_sb[:, o, :]

            def consume(ps, st):
                nc.vector.tensor_copy(out=vp_sb[:, st, 0:64], in_=ps[:, 0:64])
                nc.vector.tensor_copy(out=vp_sb[:, st, 65:129], in_=ps[:, 64:128])

            return accum(range(16), 16, get_lhsT, get_rhs, alloc, consume,
                         prep=prep, width=128, paired=False, unit=2)

        # prefetch the first K and V chunks (their projections and the rest
        # of the V/K work run as filler inside the attention stream)
        kh_prefetch = {}
        vh_prefetch = {}

        def load_kh(ns):
            kh = vkin.tile([P, 16, 512], bf, tag="kh", name="kh")
            for o4 in range(0, 16, 4):
                nc.sync.dma_start(
                    kh[:, o4 : o4 + 4, :],
                    kT3[:, o4 : o4 + 4, ns * 512 : (ns + 1) * 512],
                )
            kh_prefetch[ns] = kh

        def load_vh(sc):
            vh = vkin.tile([P, 16, 512], bf, tag="vh", name="vh")
            for o4 in range(0, 16, 4):
                nc.sync.dma_start(
                    vh[:, o4 : o4 + 4, :],
                    vT3[:, o4 : o4 + 4, sc * 512 : (sc + 1) * 512],
                )
            vh_prefetch[sc] = vh

        load_kh(0)

        # ================= prologue: K projection + RoPE =================
        def k_phase(chunks):
            def prep(ns):
                for c in (ns, ns + 1):
                    if c < 4 and c not in kh_prefetch:
                        load_kh(c)

            def alloc(ns):
                return ppsum.tile([P, 1024], f32, tag="pp", name="psk")

            def get_lhsT(ns, o):
                return wkt_sb[:, o, :]

            def get_rhs(ns, o):
                return kh_prefetch[ns][:, o, :]

            def consume(ps, ns):
                gs = slice(ns * 512, (ns + 1) * 512)
                rope(ps, gs, kpt_b[:, gs])

            return accum(chunks, 16, get_lhsT, get_rhs, alloc, consume,
                         prep=prep, paired=False, unit=2)

        # ================= Q projection (one quarter) =================
        qpt_tiles = {}

        def load_qh(quarter):
            qh_sb = qin.tile([P, 16, 512], bf, tag="qin", name="qh")
            for o4 in range(0, 16, 4):
                nc.sync.dma_start(
                    qh_sb[:, o4 : o4 + 4, :],
                    qT3[:, o4 : o4 + 4, quarter * 512 : (quarter + 1) * 512],
                )
            return qh_sb

        def qproj_gen(quarter, qh_sb):
            gs = slice(quarter * 512, (quarter + 1) * 512)
            qpt_tiles[quarter] = qpt_pool.tile([P, 4, 512], bf, tag="qpt", name="qpt_q")

            def alloc(m):
                return ppsum.tile([P, 1024], f32, tag="pp", name="psq")

            def get_lhsT(m, o):
                return wqt_sb[:, o, m * 128 : (m + 1) * 128]

            def get_rhs(m, o):
                return qh_sb[:, o, :]

            def consume(ps, m):
                rope(ps, gs, qpt_tiles[quarter][:, m, :])

            return accum(range(4), 16, get_lhsT, get_rhs, alloc, consume,
                         paired=False, unit=2)

        # ================= output projection (one quarter) =================
        outT_tiles = {}

        def outproj_gen(quarter):
            combos = [(qi, dn) for qi in range(4) for dn in range(4)]
            outT_q = outT_tiles[quarter]

            def alloc(c):
                return ppsum.tile([P, 1024], f32, tag="pp", name="psf")

            def get_lhsT(c, o):
                qi, dn = c
                return outT_q[:, o, qi * 128 : (qi + 1) * 128]

            def get_rhs(c, o):
                qi, dn = c
                return wo_sb[:, o, dn * 512 : (dn + 1) * 512]

            def consume(ps, c):
                qi, dn = c
                of = fout.tile([P, 512], f32, tag="of", name="of")
                nc.vector.tensor_copy(out=of[:], in_=ps[:, 0:512])
                nc.sync.dma_start(
                    out3[:, quarter * 4 + qi, dn * 512 : (dn + 1) * 512], of[:]
                )

            return accum(combos, 4, get_lhsT, get_rhs, alloc, consume,
                         paired=False, unit=2)

        # ---- prologue: only K chunk 0 and quarter-0 Q projection run up
        # front (minimum needed for the first scores); V and the remaining K
        # chunks stream in as filler during early attention ----
        qh0 = load_qh(0)
        load_vh(0)
        load_kh(1)
        load_vh(1)
        nc.sync.dma_start(wo_sb[:], wo3[:])
        pro = [k_phase([0, 1, 2, 3]), v_phase(), qproj_gen(0, qh0)]
        weights = [1, 2, 1]
        while pro:
            for g, w in zip(list(pro), list(weights)):
                try:
                    for _ in range(w):
                        next(g)
                except StopIteration:
                    i = pro.index(g)
                    pro.pop(i)
                    weights.pop(i)

        # ================= main loop: flattened attention pipeline =========
        from collections import deque

        fillerq = deque()

        def pop_filler():
            while fillerq:
                try:
                    next(fillerq[0])
                    return
                except StopIteration:
                    fillerq.popleft()

        av_tiles = {}

        def av_step(q, pr, pet, pkt, last):
            first = pkt == 0
            if first:
                av0 = apsum.tile([65, 512], f32, tag="av", name="av0")
                av1 = apsum.tile([65, 512], f32, tag="av", name="av1")
                av_tiles[(q, pr)] = (av0, av1)
            av0, av1 = av_tiles[(q, pr)]
            nc.tensor.matmul(
                av0, lhsT=vp_sb[:, pkt, 0:65], rhs=pet[:, 0:512],
                start=first, stop=last,
            )
            nc.tensor.matmul(
                av1, lhsT=vp_sb[:, pkt, 65:130], rhs=pet[:, 512:1024],
                start=first, stop=last,
            )
            if last:
                outT_q = outT_tiles[q]
                avcs = []
                for av in (av0, av1):
                    avc = ntmp.tile([65, 512], f32, tag="avc", name="avc")
                    nc.vector.tensor_copy(out=avc[:], in_=av[:])
                    avcs.append(avc)
                for e, avc in enumerate(avcs):
                    recip = ntmp.tile([1, 512], f32, tag="recip", name="recip")
                    nc.vector.reciprocal(recip[:], avc[64:65, :])
                    bc = ntmp.tile([64, 512], f32, tag="bc", name="bc")
                    nc.gpsimd.partition_broadcast(bc[:], recip[:])
                    hp = slice(e * 64, e * 64 + 64)
                    nc.vector.tensor_mul(
                        out=outT_q[hp, pr, :], in0=avcs[e][0:64, :], in1=bc[:]
                    )

        steps = [(q, pr, kt) for q in range(4) for pr in range(4)
                 for kt in range(16)]
        prev = None
        for q, pr, kt in steps:
            if pr == 0 and kt == 0:
                outT_q = outT_pool.tile([P, 4, 512], bf, tag="outT",
                                        name="outT_q")
                outT_tiles[q] = outT_q
                if q < 3:
                    qh_next = load_qh(q + 1)
                    fillerq.append(qproj_gen(q + 1, qh_next))
            if pr == 1 and kt == 0 and q > 0:
                # deferred so outT(q-1)'s last norm (emitted during pair 0's
                # first steps) exists before any outproj unit references it
                fillerq.append(outproj_gen(q - 1))

            qpt_q = qpt_tiles[q]
            quad = qpsum.tile([P, 1024], f32, tag="qp", name="quad")
            ksl = slice(kt * 128, (kt + 1) * 128)
            nc.tensor.matmul(
                quad[:, 0:512], lhsT=kpt_b[LO, ksl],
                rhs=qpt_q[LO, pr, :], start=True, stop=True,
            )
            nc.tensor.matmul(
                quad[:, 512:1024], lhsT=kpt_b[HI, ksl],
                rhs=qpt_q[HI, pr, :], start=True, stop=True,
            )
            et = etp.tile([P, 1024], bf, tag="et", name="et")
            nc.scalar.activation(
                out=et[:], in_=quad[:], func=Exp, scale=scale
            )
            # pops come BEFORE the AV step: the V/K filler units produce the
            # vp/kpt tiles that AV and scores consume, and a filler matmul
            # queued behind a stalled AV would deadlock the PE FIFO
            pop_filler()
            if prev is not None:
                pq, ppr, pet, pkt = prev
                av_step(pq, ppr, pet, pkt, last=(pkt == 15))
            prev = (q, pr, et, kt)
        # drain the last AV step and remaining filler
        pq, ppr, pet, pkt = prev
        av_step(pq, ppr, pet, pkt, last=True)
        while fillerq:
            try:
                next(fillerq[0])
            except StopIteration:
                fillerq.popleft()

        # epilogue: out projection of last quarter
        run_all(outproj_gen(3))

    nc.finalize()
    return nc


def _host_inputs(q, k, v, Wq, Wk, Wv, Wo):
    """Build the 8 per-core input dicts."""
    inv_freq = 1.0 / (THETA ** (np.arange(0, HD, 2, dtype=np.float32) / HD))
    t = np.arange(S, dtype=np.float32)
    freqs = np.einsum("i,j->ij", t, inv_freq)
    emb = np.concatenate([freqs, freqs], axis=-1)  # [S, 64]
    cosT = np.ascontiguousarray(np.cos(emb).T, dtype=np.float32)  # [64, S]
    sinT = np.ascontiguousarray(np.sin(emb).T, dtype=np.float32)
    cos_rep = np.concatenate([cosT, cosT], axis=0).astype(BF16)  # [128, S]
    sin_rep = np.concatenate([sinT, sinT], axis=0).astype(BF16)

    qT = [np.ascontiguousarray(q[b].T).astype(BF16) for b in range(B)]
    kTt = [np.ascontiguousarray(k[b].T).astype(BF16) for b in range(B)]
    vTt = [np.ascontiguousarray(v[b].T).astype(BF16) for b in range(B)]

    in_maps = []
    for c in range(NCORES):
        b, g = divmod(c, 4)
        # pair-interleaved: chunk i of 128 cols = (kv0 q-head i, kv1 q-head i)
        qheads = [2 * g, 2 * g + 1, 2 * g + 8, 2 * g + 9,
                  2 * g + 16, 2 * g + 17, 2 * g + 24, 2 * g + 25]
        qcols = np.concatenate([np.arange(h * HD, (h + 1) * HD) for h in qheads])
        kvcols = np.arange(2 * g * HD, (2 * g + 2) * HD)

        wqt_np = np.ascontiguousarray(Wq[:, qcols]).astype(BF16)
        wkt_np = np.ascontiguousarray(Wk[:, kvcols]).astype(BF16)
        wv_np = np.ascontiguousarray(Wv[:, kvcols]).astype(BF16)
        wo_np = np.ascontiguousarray(Wo[qcols, :]).astype(BF16)

        in_maps.append({
            "qT": qT[b], "kT": kTt[b], "vT": vTt[b],
            "wqt": wqt_np, "wkt": wkt_np, "wv": wv_np, "wo": wo_np,
            "cosr": cos_rep, "sinr": sin_rep,
        })
    return in_maps


def kernel(q, k, v, attn_mask, Wq, Wk, Wv, Wo, bo):
    from concourse.bass_utils import run_bass_kernel_spmd

    q = np.asarray(q, dtype=np.float32)
    k = np.asarray(k, dtype=np.float32)
    v = np.asarray(v, dtype=np.float32)
    Wq = np.asarray(Wq, dtype=np.float32)
    Wk = np.asarray(Wk, dtype=np.float32)
    Wv = np.asarray(Wv, dtype=np.float32)
    Wo = np.asarray(Wo, dtype=np.float32)
    bo = np.asarray(bo, dtype=np.float32)

    if "nc" not in _CACHE:
        _CACHE["nc"] = _build_program()
    nc = _CACHE["nc"]

    in_maps = _host_inputs(q, k, v, Wq, Wk, Wv, Wo)
    trace = bool(int(os.environ.get("KERNEL_TRACE", "0")))
    res = run_bass_kernel_spmd(nc, in_maps, core_ids=list(range(NCORES)),
                               trace=trace)
    _CACHE["last_result"] = res

    out = np.zeros((B, S, D), dtype=np.float32)
    for c in range(NCORES):
        b = c // 4
        out[b] += np.asarray(res.results[c]["out"], dtype=np.float32)
    out += bo[None, None, :]
    return out


# revision 18
# speedup vs baseline: 1.1386x; 1.1343x over previous
"""GQA attention block on 8 trn2 NeuronCores.

Sharding: core c = (batch b=c//4, kv-head-pair g=c%4). Each core owns kv heads
{2g, 2g+1} and their 8 query heads (GQA tile mapping: q-head i -> kv-head i%8),
with Wq/Wk/Wv column-sharded and Wo row-sharded; host sums the 4 partial
outputs per batch and adds bo.

Device strategy (per core):
  - scores: the two heads of a pair run CONCURRENTLY as 64x128 PE row tiles
    (T0 reads SBUF partitions 0-63 = even head, T8 reads 64-127 = odd head),
    each writing its own PSUM bank of a shared [128,1024] f32 quad (a PSUM
    bank must never be written by two row tiles concurrently).
  - exp on ACT in 1024-wide chunks (amortizes the ~352-cycle ACTIVATE
    overhead), scale=1/8 folded in, bf16 out.
  - AV and all projections run as plain full-array 128x128 matmuls (single
    writer per PSUM bank). AV: lhsT = Vp chunk [128 kpos, 65] with a ones
    column giving the softmax denominator in psum row 64.
  - Q/O projection work is emitted as fine-grained "filler" units between
    attention steps so the PE stays busy while ACT chews exp.
  - RoPE: PSUM evacuated to bf16 SBUF, rotate_half via partition-shifted DVE
    copies, cos/sin combine in bf16 (fast DVE modes).
"""

import os
from contextlib import ExitStack

import numpy as np
import ml_dtypes

D = 2048
QH = 32
KVH = 8
HD = 64
B = 2
S = 2048
THETA = 1000000.0
P = 128
NCORES = 8

BF16 = ml_dtypes.bfloat16

_CACHE = {}


def _build_program():
    import concourse.bass as bass
    import concourse.tile as tile
    from concourse import bacc, mybir

    nc = bacc.Bacc(
        "TRN2",
        target_bir_lowering=False,
        debug=False,
        enable_asserts=False,
        num_devices=NCORES,
    )
    bf = mybir.dt.bfloat16
    f32 = mybir.dt.float32

    qT = nc.dram_tensor("qT", [D, S], bf, kind="ExternalInput").ap()
    kT = nc.dram_tensor("kT", [D, S], bf, kind="ExternalInput").ap()
    vT = nc.dram_tensor("vT", [D, S], bf, kind="ExternalInput").ap()
    wqt = nc.dram_tensor("wqt", [D, 512], bf, kind="ExternalInput").ap()
    wkt = nc.dram_tensor("wkt", [D, 128], bf, kind="ExternalInput").ap()
    wv = nc.dram_tensor("wv", [D, 128], bf, kind="ExternalInput").ap()
    wo = nc.dram_tensor("wo", [512, D], bf, kind="ExternalInput").ap()
    cosr = nc.dram_tensor("cosr", [P, S], bf, kind="ExternalInput").ap()
    sinr = nc.dram_tensor("sinr", [P, S], bf, kind="ExternalInput").ap()
    out = nc.dram_tensor("out", [S, D], f32, kind="ExternalOutput").ap()

    # partitioned DRAM views
    qT3 = qT.rearrange("(o p) s -> p o s", p=P)    # [128, 16, 2048]
    kT3 = kT.rearrange("(o p) s -> p o s", p=P)
    vT3 = vT.rearrange("(o p) s -> p o s", p=P)
    wqt3 = wqt.rearrange("(o p) m -> p o m", p=P)  # [128, 16, 512]
    wkt3 = wkt.rearrange("(o p) m -> p o m", p=P)  # [128, 16, 128]
    wv3 = wv.rearrange("(o p) m -> p o m", p=P)    # [128, 16, 128]
    wo3 = wo.rearrange("(o p) d -> p o d", p=P)    # [128, 4, 2048]
    out3 = out.rearrange("(t p) d -> p t d", p=P)  # [128, 16, 2048]

    scale = 1.0 / float(np.sqrt(HD))
    LO = slice(0, 64)
    HI = slice(64, 128)

    with tile.TileContext(nc) as tc, ExitStack() as ctx:
        Exp = mybir.ActivationFunctionType.Exp
        const = ctx.enter_context(tc.tile_pool(name="const", bufs=1))
        persist = ctx.enter_context(tc.tile_pool(name="persist", bufs=1))
        qpt_pool = ctx.enter_context(tc.tile_pool(name="qptp", bufs=2))
        outT_pool = ctx.enter_context(tc.tile_pool(name="outTp", bufs=2))
        vkin = ctx.enter_context(tc.tile_pool(name="vkin", bufs=2))
        qin = ctx.enter_context(tc.tile_pool(name="qin", bufs=2))
        rtmp = ctx.enter_context(tc.tile_pool(name="rtmp", bufs=2))
        fout = ctx.enter_context(tc.tile_pool(name="fout", bufs=5))
        ntmp = ctx.enter_context(tc.tile_pool(name="ntmp", bufs=2))
        etp = ctx.enter_context(tc.tile_pool(name="etp", bufs=3))
        qpsum = ctx.enter_context(tc.tile_pool(name="qpsum", bufs=2, space="PSUM"))
        apsum = ctx.enter_context(tc.tile_pool(name="apsum", bufs=2, space="PSUM"))
        ppsum = ctx.enter_context(tc.tile_pool(name="ppsum", bufs=2, space="PSUM"))

        # ---- resident weights / tables (small V/K weights first so the
        # V projection can start while the big tables stream in) ----
        wv_sb = const.tile([P, 16, 128], bf, tag="wv")
        nc.sync.dma_start(wv_sb[:], wv3[:])
        wkt_sb = const.tile([P, 16, 128], bf, tag="wkt")
        nc.sync.dma_start(wkt_sb[:], wkt3[:])
        cos_sb = const.tile([P, S], bf, tag="cos")
        nc.sync.dma_start(cos_sb[:], cosr[:])
        sin_sb = const.tile([P, S], bf, tag="sin")
        nc.sync.dma_start(sin_sb[:], sinr[:])
        wqt_sb = const.tile([P, 16, 512], bf, tag="wqt")
        nc.sync.dma_start(wqt_sb[:], wqt3[:])
        wo_sb = const.tile([P, 4, 2048], bf, tag="wo")

        # ---- persistent intermediates ----
        kpt_b = persist.tile([P, S], bf, tag="kpt")       # rotated K^T pair-stacked
        vp_sb = persist.tile([P, 16, 130], bf, tag="vp")  # Vp + ones cols
        nc.vector.memset(vp_sb[:, :, 64:65], 1.0)
        nc.vector.memset(vp_sb[:, :, 129:130], 1.0)

        def rope(ps, gs, dst):
            """RoPE: ps [128,512] f32 psum (pair-stacked head dims) ->
            dst bf16 [128,512]. Evacuate early to free the bank, then bf16
            DVE ops (rotate_half = partition-shifted copies)."""
            ev = rtmp.tile([P, 512], bf, tag="ev", name="ev")
            nc.vector.tensor_copy(out=ev[:], in_=ps[:, 0:512])
            rot = rtmp.tile([P, 512], bf, tag="rot", name="rot")
            for b0 in (0, 64):
                nc.vector.tensor_scalar_mul(
                    rot[b0 : b0 + 32, :], ev[b0 + 32 : b0 + 64, :], -1.0
                )
                nc.vector.tensor_copy(
                    out=rot[b0 + 32 : b0 + 64, :], in_=ev[b0 : b0 + 32, :]
                )
            t1 = rtmp.tile([P, 512], bf, tag="t1", name="t1")
            t2 = rtmp.tile([P, 512], bf, tag="t2", name="t2")
            nc.vector.tensor_mul(out=t1[:], in0=ev[:], in1=cos_sb[:, gs])
            nc.vector.tensor_mul(out=t2[:], in0=rot[:], in1=sin_sb[:, gs])
            nc.vector.tensor_add(out=dst, in0=t1[:], in1=t2[:])

        def accum(chunks, nacc, get_lhsT, get_rhs, alloc, consume,
                  prep=None, unit=4, width=None, paired=True):
            """Generator emitting row-tile-paired PSUM accumulation chains.

            For each chunk, tile T0 (SBUF partitions 0-63) accumulates the
            low contraction halves into psum columns [0:N] (bank group A)
            while T8 (partitions 64-127) concurrently accumulates the high
            halves into columns [N:2N] (bank group B). Each bank has a
            single writer; the consumer merges A+B. Yields every `unit`
            matmuls."""
            cnt = 0
            for desc in list(chunks):
                if prep is not None:
                    prep(desc)
                ps = alloc(desc)
                w = width if width is not None else 512
                for o in range(nacc):
                    lhsT = get_lhsT(desc, o)
                    rhs = get_rhs(desc, o)
                    if paired:
                        nc.tensor.matmul(
                            ps[:, 0:w], lhsT=lhsT[LO], rhs=rhs[LO],
                            start=(o == 0), stop=(o == nacc - 1),
                        )
                        nc.tensor.matmul(
                            ps[:, 512 : 512 + w], lhsT=lhsT[HI], rhs=rhs[HI],
                            start=(o == 0), stop=(o == nacc - 1),
                        )
                        cnt += 2
                    else:
                        nc.tensor.matmul(
                            ps[:, 0:w], lhsT=lhsT, rhs=rhs,
                            start=(o == 0), stop=(o == nacc - 1),
                        )
                        cnt += 1
                    if cnt >= unit:
                        cnt = 0
                        yield
                consume(ps, desc)

        def run_all(gen):
            for _ in gen:
                pass

        # ================= prologue: V projection =================
        # vp[s,128] per s-tile via stationary vT s-tiles, moving wv.
        def v_phase():

            def prep(st):
                if st % 4 != 0:
                    return
                sc = st // 4
                for c in (sc, sc + 1):
                    if c < 4 and c not in vh_prefetch:
                        load_vh(c)

            def alloc(st):
                return qpsum.tile([P, 1024], f32, tag="qp", name="psv")

            def get_lhsT(st, o):
                return vh_prefetch[st // 4][
                    :, o, (st % 4) * 128 : (st % 4 + 1) * 128
                ]

            def get_rhs(st, o):
                return wv_sb[:, o, :]

            def consume(ps, st):
                nc.vector.tensor_copy(out=vp_sb[:, st, 0:64], in_=ps[:, 0:64])
                nc.vector.tensor_copy(out=vp_sb[:, st, 65:129], in_=ps[:, 64:128])

            return accum(range(16), 16, get_lhsT, get_rhs, alloc, consume,
                         prep=prep, width=128, paired=False, unit=2)

        # prefetch the first K and V chunks (their projections and the rest
        # of the V/K work run as filler inside the attention stream)
        kh_prefetch = {}
        vh_prefetch = {}

        def load_kh(ns):
            kh = vkin.tile([P, 16, 512], bf, tag="kh", name="kh")
            for o4 in range(0, 16, 4):
                nc.sync.dma_start(
                    kh[:, o4 : o4 + 4, :],
                    kT3[:, o4 : o4 + 4, ns * 512 : (ns + 1) * 512],
                )
            kh_prefetch[ns] = kh

        def load_vh(sc):
            vh = vkin.tile([P, 16, 512], bf, tag="vh", name="vh")
            for o4 in range(0, 16, 4):
                nc.sync.dma_start(
                    vh[:, o4 : o4 + 4, :],
                    vT3[:, o4 : o4 + 4, sc * 512 : (sc + 1) * 512],
                )
            vh_prefetch[sc] = vh

        load_kh(0)

        # ================= prologue: K projection + RoPE =================
        def k_phase(chunks):
            def prep(ns):
                for c in (ns, ns + 1):
                    if c < 4 and c not in kh_prefetch:
                        load_kh(c)

            def alloc(ns):
                return ppsum.tile([P, 512], f32, tag="pp", name="psk")

            def get_lhsT(ns, o):
                return wkt_sb[:, o, :]

            def get_rhs(ns, o):
                return kh_prefetch[ns][:, o, :]

            def consume(ps, ns):
                gs = slice(ns * 512, (ns + 1) * 512)
                rope(ps, gs, kpt_b[:, gs])

            return accum(chunks, 16, get_lhsT, get_rhs, alloc, consume,
                         prep=prep, paired=False, unit=2)

        # ================= Q projection (one quarter) =================
        qpt_tiles = {}

        def load_qh(quarter):
            qh_sb = qin.tile([P, 16, 512], bf, tag="qin", name="qh")
            for o4 in range(0, 16, 4):
                nc.sync.dma_start(
                    qh_sb[:, o4 : o4 + 4, :],
                    qT3[:, o4 : o4 + 4, quarter * 512 : (quarter + 1) * 512],
                )
            return qh_sb

        def qproj_gen(quarter, qh_sb):
            gs = slice(quarter * 512, (quarter + 1) * 512)
            qpt_tiles[quarter] = qpt_pool.tile([P, 4, 512], bf, tag="qpt", name="qpt_q")

            def alloc(m):
                return ppsum.tile([P, 512], f32, tag="pp", name="psq")

            def get_lhsT(m, o):
                return wqt_sb[:, o, m * 128 : (m + 1) * 128]

            def get_rhs(m, o):
                return qh_sb[:, o, :]

            def consume(ps, m):
                rope(ps, gs, qpt_tiles[quarter][:, m, :])

            return accum(range(4), 16, get_lhsT, get_rhs, alloc, consume,
                         paired=False, unit=2)

        # ================= output projection (one quarter) =================
        outT_tiles = {}

        def outproj_gen(quarter):
            combos = [(qi, dn) for qi in range(4) for dn in range(4)]
            outT_q = outT_tiles[quarter]

            def alloc(c):
                return ppsum.tile([P, 512], f32, tag="pp", name="psf")

            def get_lhsT(c, o):
                qi, dn = c
                return outT_q[:, o, qi * 128 : (qi + 1) * 128]

            def get_rhs(c, o):
                qi, dn = c
                return wo_sb[:, o, dn * 512 : (dn + 1) * 512]

            def consume(ps, c):
                qi, dn = c
                of = fout.tile([P, 512], f32, tag="of", name="of")
                nc.vector.tensor_copy(out=of[:], in_=ps[:, 0:512])
                nc.sync.dma_start(
                    out3[:, quarter * 4 + qi, dn * 512 : (dn + 1) * 512], of[:]
                )

            return accum(combos, 4, get_lhsT, get_rhs, alloc, consume,
                         paired=False, unit=2)

        # ---- prologue: only K chunk 0 and quarter-0 Q projection run up
        # front (minimum needed for the first scores); V and the remaining K
        # chunks stream in as filler during early attention ----
        qh0 = load_qh(0)
        load_vh(0)
        load_kh(1)
        load_vh(1)
        nc.sync.dma_start(wo_sb[:], wo3[:])
        pro = [k_phase([0, 1, 2, 3]), v_phase(), qproj_gen(0, qh0)]
        weights = [1, 2, 1]
        while pro:
            for g, w in zip(list(pro), list(weights)):
                try:
                    for _ in range(w):
                        next(g)
                except StopIteration:
                    i = pro.index(g)
                    pro.pop(i)
                    weights.pop(i)

        # ================= main loop: flattened attention pipeline =========
        from collections import deque

        fillerq = deque()

        def pop_filler():
            while fillerq:
                try:
                    next(fillerq[0])
                    return
                except StopIteration:
                    fillerq.popleft()

        av_tiles = {}

        def av_step(q, pr, pet, pkt, last):
            first = pkt == 0
            if first:
                av0 = apsum.tile([65, 512], f32, tag="av", name="av0")
                av1 = apsum.tile([65, 512], f32, tag="av", name="av1")
                av_tiles[(q, pr)] = (av0, av1)
            av0, av1 = av_tiles[(q, pr)]
            nc.tensor.matmul(
                av0, lhsT=vp_sb[:, pkt, 0:65], rhs=pet[:, 0:512],
                start=first, stop=last,
            )
            nc.tensor.matmul(
                av1, lhsT=vp_sb[:, pkt, 65:130], rhs=pet[:, 512:1024],
                start=first, stop=last,
            )
            if last:
                outT_q = outT_tiles[q]
                avcs = []
                for av in (av0, av1):
                    avc = ntmp.tile([65, 512], f32, tag="avc", name="avc")
                    nc.vector.tensor_copy(out=avc[:], in_=av[:])
                    avcs.append(avc)
                for e, avc in enumerate(avcs):
                    recip = ntmp.tile([1, 512], f32, tag="recip", name="recip")
                    nc.vector.reciprocal(recip[:], avc[64:65, :])
                    bc = ntmp.tile([64, 512], f32, tag="bc", name="bc")
                    nc.gpsimd.partition_broadcast(bc[:], recip[:])
                    hp = slice(e * 64, e * 64 + 64)
                    nc.vector.tensor_mul(
                        out=outT_q[hp, pr, :], in0=avcs[e][0:64, :], in1=bc[:]
                    )

        steps = [(q, pr, kt) for q in range(4) for pr in range(4)
                 for kt in range(16)]
        prev = None
        for q, pr, kt in steps:
            if pr == 0 and kt == 0:
                outT_q = outT_pool.tile([P, 4, 512], bf, tag="outT",
                                        name="outT_q")
                outT_tiles[q] = outT_q
                if q < 3:
                    qh_next = load_qh(q + 1)
                    fillerq.append(qproj_gen(q + 1, qh_next))
            if pr == 1 and kt == 0 and q > 0:
                # deferred so outT(q-1)'s last norm (emitted during pair 0's
                # first steps) exists before any outproj unit references it
                fillerq.append(outproj_gen(q - 1))

            qpt_q = qpt_tiles[q]
            quad = qpsum.tile([P, 1024], f32, tag="qp", name="quad")
            ksl = slice(kt * 128, (kt + 1) * 128)
            nc.tensor.matmul(
                quad[:, 0:512], lhsT=kpt_b[LO, ksl],
                rhs=qpt_q[LO, pr, :], start=True, stop=True,
            )
            nc.tensor.matmul(
                quad[:, 512:1024], lhsT=kpt_b[HI, ksl],
                rhs=qpt_q[HI, pr, :], start=True, stop=True,
            )
            et = etp.tile([P, 1024], bf, tag="et", name="et")
            nc.scalar.activation(
                out=et[:], in_=quad[:], func=Exp, scale=scale
            )
            # pops come BEFORE the AV step: the V/K filler units produce the
            # vp/kpt tiles that AV and scores consume, and a filler matmul
            # queued behind a stalled AV would deadlock the PE FIFO
            pop_filler()
            if prev is not None:
                pq, ppr, pet, pkt = prev
                av_step(pq, ppr, pet, pkt, last=(pkt == 15))
            prev = (q, pr, et, kt)
        # drain the last AV step and remaining filler
        pq, ppr, pet, pkt = prev
        av_step(pq, ppr, pet, pkt, last=True)
        while fillerq:
            try:
                next(fillerq[0])
            except StopIteration:
                fillerq.popleft()

        # epilogue: out projection of last quarter
        run_all(outproj_gen(3))

    nc.finalize()
    return nc


def _host_inputs(q, k, v, Wq, Wk, Wv, Wo):
    """Build the 8 per-core input dicts."""
    inv_freq = 1.0 / (THETA ** (np.arange(0, HD, 2, dtype=np.float32) / HD))
    t = np.arange(S, dtype=np.float32)
    freqs = np.einsum("i,j->ij", t, inv_freq)
    emb = np.concatenate([freqs, freqs], axis=-1)  # [S, 64]
    cosT = np.ascontiguousarray(np.cos(emb).T, dtype=np.float32)  # [64, S]
    sinT = np.ascontiguousarray(np.sin(emb).T, dtype=np.float32)
    cos_rep = np.concatenate([cosT, cosT], axis=0).astype(BF16)  # [128, S]
    sin_rep = np.concatenate([sinT, sinT], axis=0).astype(BF16)

    qT = [np.ascontiguousarray(q[b].T).astype(BF16) for b in range(B)]
    kTt = [np.ascontiguousarray(k[b].T).astype(BF16) for b in range(B)]
    vTt = [np.ascontiguousarray(v[b].T).astype(BF16) for b in range(B)]

    in_maps = []
    for c in range(NCORES):
        b, g = divmod(c, 4)
        # pair-interleaved: chunk i of 128 cols = (kv0 q-head i, kv1 q-head i)
        qheads = [2 * g, 2 * g + 1, 2 * g + 8, 2 * g + 9,
                  2 * g + 16, 2 * g + 17, 2 * g + 24, 2 * g + 25]
        qcols = np.concatenate([np.arange(h * HD, (h + 1) * HD) for h in qheads])
        kvcols = np.arange(2 * g * HD, (2 * g + 2) * HD)

        wqt_np = np.ascontiguousarray(Wq[:, qcols]).astype(BF16)
        wkt_np = np.ascontiguousarray(Wk[:, kvcols]).astype(BF16)
        wv_np = np.ascontiguousarray(Wv[:, kvcols]).astype(BF16)
        wo_np = np.ascontiguousarray(Wo[qcols, :]).astype(BF16)

        in_maps.append({
            "qT": qT[b], "kT": kTt[b], "vT": vTt[b],
            "wqt": wqt_np, "wkt": wkt_np, "wv": wv_np, "wo": wo_np,
            "cosr": cos_rep, "sinr": sin_rep,
        })
    return in_maps


def kernel(q, k, v, attn_mask, Wq, Wk, Wv, Wo, bo):
    from concourse.bass_utils import run_bass_kernel_spmd

    q = np.asarray(q, dtype=np.float32)
    k = np.asarray(k, dtype=np.float32)
    v = np.asarray(v, dtype=np.float32)
    Wq = np.asarray(Wq, dtype=np.float32)
    Wk = np.asarray(Wk, dtype=np.float32)
    Wv = np.asarray(Wv, dtype=np.float32)
    Wo = np.asarray(Wo, dtype=np.float32)
    bo = np.asarray(bo, dtype=np.float32)

    if "nc" not in _CACHE:
        _CACHE["nc"] = _build_program()
    nc = _CACHE["nc"]

    in_maps = _host_inputs(q, k, v, Wq, Wk, Wv, Wo)
    trace = bool(int(os.environ.get("KERNEL_TRACE", "0")))
    res = run_bass_kernel_spmd(nc, in_maps, core_ids=list(range(NCORES)),
                               trace=trace)
    _CACHE["last_result"] = res

    out = np.zeros((B, S, D), dtype=np.float32)
    for c in range(NCORES):
        b = c // 4
        out[b] += np.asarray(res.results[c]["out"], dtype=np.float32)
    out += bo[None, None, :]
    return out
